# revision 1
# baseline (speedup 1.0000x reference)
"""Multi-head self-attention with SDPA softcap, sharded over 8 NeuronCores.

Sharding: tensor-parallel over heads. Each core owns 2 of the 16 heads:
  - computes q,k,v projections for its head slice (fp32r matmuls),
  - does attention (softcap tanh + softmax) for its heads over both batches,
  - applies its row-slice of the output projection, producing a partial
    [B*S, D] output. Host sums the 8 partials.

All heavy matmuls run in float32r (fp32 with 11-bit mantissa, full PE rate).
Inputs are pre-rounded to fp32r on the host so device rounding is exact.
"""

import sys

if "/opt/trn_rl_repo" not in sys.path:
    sys.path.insert(0, "/opt/trn_rl_repo")

import numpy as np

import concourse.bass as bass
import concourse.bacc as bacc
import concourse.tile as tile
from concourse import mybir
from concourse.bass_utils import run_bass_kernel_spmd
from concourse.masks import make_identity

F32 = mybir.dt.float32
F32R = mybir.dt.float32r

D = 2048          # model dim
H = 16            # total heads
DK = 128          # head dim
B = 2
S = 2048
T = B * S         # 4096 total tokens
NCORES = 8
HC = 2            # heads per core
DPC = HC * DK     # 256: d' slice per core

KC = D // 128     # 16 contraction chunks over model dim
TCOL = 512        # phase-1 token-column width
NTCOL = T // TCOL             # 16
TQ = 256          # phase-2 query-column width
NTQ = S // TQ                 # 8 per batch
NTK = S // 128    # 16 key blocks per batch


def _round_fp32r(x: np.ndarray) -> np.ndarray:
    """Round fp32 to fp32r (11-bit mantissa), round-to-nearest-even."""
    u = np.ascontiguousarray(x, dtype=np.float32).view(np.uint32)
    low = u & np.uint32(0xFFF)
    kept = u & np.uint32(0xFFFFF000)
    half = np.uint32(0x800)
    roundup = (low > half) | ((low == half) & ((kept & np.uint32(0x1000)) != 0))
    out = kept + np.where(roundup, np.uint32(0x1000), np.uint32(0))
    return out.view(np.float32)

def _build_program(cap: float):
    nc = bacc.Bacc("TRN2", target_bir_lowering=False, debug=False,
                   num_devices=NCORES)

    xT = nc.dram_tensor("xT", [D, T], F32R, kind="ExternalInput").ap()
    ones_d = nc.dram_tensor("ones", [128, 128], F32R, kind="ExternalInput").ap()
    wqT = nc.dram_tensor("wqT", [D, DPC], F32R, kind="ExternalInput").ap()
    wkT = nc.dram_tensor("wkT", [D, DPC], F32R, kind="ExternalInput").ap()
    wvT = nc.dram_tensor("wvT", [D, DPC], F32R, kind="ExternalInput").ap()
    woT = nc.dram_tensor("woT", [DPC, D], F32R, kind="ExternalInput").ap()
    biasT = nc.dram_tensor("biasT", [S, S], F32, kind="ExternalInput").ap()
    out_d = nc.dram_tensor("out_partial", [T, D], F32, kind="ExternalOutput").ap()

    xT_v = xT.rearrange("(kc p) t -> p kc t", p=128)
    biasT_v = biasT.rearrange("(kc p) t -> p kc t", p=128)

    NB = S // 128       # 16 key blocks per batch
    NHF = NTK // 2      # tanh/exp half size (8 key blocks)
    KH = KC // 2

    with tile.TileContext(nc) as tc:
        with (
            tc.tile_pool(name="const", bufs=1) as cpool,
            tc.tile_pool(name="dscr", bufs=1, space="DRAM") as dscr,
            tc.tile_pool(name="wide", bufs=2, space="PSUM") as wide,
            tc.tile_pool(name="spsp", bufs=2, space="PSUM") as spsp,
            tc.tile_pool(name="acc", bufs=2, space="PSUM") as acc,
            tc.tile_pool(name="p2kv", bufs=1) as p2kv,
            tc.tile_pool(name="pqw", bufs=1) as pqw,
        ):
            kT_dram = dscr.tile([HC, 128, T], F32R)       # [h, dk, t]
            v_dram = dscr.tile([T // 128, 128, HC * DK], F32R)  # [tkb, tk%, (h d')]

            ident = cpool.tile([128, 128], F32)
            make_identity(nc, ident[:])
            ones_full = cpool.tile([128, 128], F32R)
            nc.sync.dma_start(out=ones_full[:], in_=ones_d[:])
            wq_sb = pqw.tile([128, KC, DPC], F32R)
            nc.scalar.dma_start(
                out=wq_sb[:], in_=wqT.rearrange("(kc p) n -> p kc n", p=128))

            kv_cur = {}

            def load_kv(b):
                kb = p2kv.tile([128, HC, S], F32R, tag="kb")
                vb = p2kv.tile([128, HC, NB, DK], F32R, tag="vb")
                for h in range(HC):
                    nc.scalar.dma_start(
                        out=kb[:, h, :],
                        in_=kT_dram[h, :, b * S:(b + 1) * S],
                    )
                    nc.scalar.dma_start(
                        out=vb[:, h, :, :],
                        in_=v_dram[b * NB:(b + 1) * NB, :,
                                   h * DK:(h + 1) * DK]
                        .rearrange("a p b -> p a b"),
                    )
                kv_cur[b] = (kb, vb)

            # ---------- Phase 1a: k and v projections (all tokens) ----------
            with (
                tc.tile_pool(name="p1w", bufs=1) as p1w,
                tc.tile_pool(name="p1x", bufs=6) as p1x,
                tc.tile_pool(name="p1tmp", bufs=3) as p1tmp,
                tc.tile_pool(name="p1v", bufs=4) as p1v,
            ):
                wk_sb = p1w.tile([128, KC, DPC], F32R)
                wv_sb = p1w.tile([128, KC, DPC], F32R)
                nc.sync.dma_start(
                    out=wk_sb[:], in_=wkT.rearrange("(kc p) n -> p kc n", p=128))
                nc.scalar.dma_start(
                    out=wv_sb[:], in_=wvT.rearrange("(kc p) n -> p kc n", p=128))

                for tcol in range(NTCOL):
                    t0 = tcol * TCOL
                    xcol_a = p1x.tile([128, KH, TCOL], F32R, tag="xcol")
                    xcol_b = p1x.tile([128, KH, TCOL], F32R, tag="xcol")
                    nc.sync.dma_start(
                        out=xcol_a[:], in_=xT_v[:, 0:KH, t0:t0 + TCOL])
                    nc.sync.dma_start(
                        out=xcol_b[:], in_=xT_v[:, KH:KC, t0:t0 + TCOL])

                    # k: stationary weights, transposed output
                    ps = wide.tile([128, HC, TCOL], F32, tag="wide")
                    for m in range(HC):
                        for kc in range(KC):
                            xc = xcol_a if kc < KH else xcol_b
                            nc.tensor.matmul(
                                ps[:, m, :],
                                wk_sb[:, kc, m * 128:(m + 1) * 128],
                                xc[:, kc % KH, :],
                                start=(kc == 0),
                                stop=(kc == KC - 1),
                            )
                    st = p1tmp.tile([128, HC, TCOL], F32R, tag="st")
                    nc.vector.tensor_copy(
                        st[:].rearrange("p a b -> p (a b)"),
                        ps[:].rearrange("p a b -> p (a b)"),
                    )
                    for m in range(HC):
                        nc.gpsimd.dma_start(
                            out=kT_dram[m, :, t0:t0 + TCOL],
                            in_=st[:, m, :],
                        )
                    # v: stationary x chunks -> natural [t, (h d')] layout
                    for tsub in range(TCOL // 128):
                        if tsub % 2 == 0:
                            vp = acc.tile([128, 512], F32, tag="acc")
                        else:
                            vp2 = spsp.tile(
                                [128, 2, TQ], F32, tag="sps", name="vp2")
                            vp = vp2[:].rearrange("p a b -> p (a b)")
                        for kc in range(KC):
                            xc = xcol_a if kc < KH else xcol_b
                            nc.tensor.matmul(
                                vp[:, 0:DPC],
                                xc[:, kc % KH, tsub * 128:(tsub + 1) * 128],
                                wv_sb[:, kc, :],
                                start=(kc == 0),
                                stop=(kc == KC - 1),
                            )
                        vst = p1v.tile([128, DPC], F32R, tag="vst")
                        nc.vector.tensor_copy(vst[:], vp[:, 0:DPC])
                        nc.gpsimd.dma_start(
                            out=v_dram[tcol * (TCOL // 128) + tsub, :, :],
                            in_=vst[:],
                        )
                    if tcol == NTCOL // 2 - 1:
                        load_kv(0)  # b=0 k/v landed; prefetch during back half

            # -------- Mixed: q projection streamed + attention + out-proj ----
            with (
                tc.tile_pool(name="pqx", bufs=2) as pqx,
                tc.tile_pool(name="pqst", bufs=3) as pqst,
                tc.tile_pool(name="p2bias", bufs=2) as p2bias,
                tc.tile_pool(name="p2s", bufs=2) as p2s,
                tc.tile_pool(name="p2er", bufs=2) as p2er,
                tc.tile_pool(name="p2misc", bufs=2) as p2misc,
                tc.tile_pool(name="p2ot", bufs=4) as p2ot,
                tc.tile_pool(name="p3w", bufs=1) as p3w,
                tc.tile_pool(name="p3out", bufs=2) as p3out,
            ):
                wo_sb = p3w.tile([128, HC, 4, 512], F32R)
                for hc in range(HC):
                    nc.scalar.dma_start(
                        out=wo_sb[:, hc, :, :],
                        in_=woT[hc * 128:(hc + 1) * 128, :].rearrange(
                            "p (nc n) -> p nc n", n=512
                        ),
                    )

                q_done = set()
                q_cols = {}
                TCQ = 256  # q-pass token-column width

                def ensure_q(b, tqc):
                    g = (b * S + tqc * TQ) // TCQ
                    if g in q_done:
                        return
                    q_done.add(g)
                    t0 = g * TCQ
                    xa = pqx.tile([128, KH, TCQ], F32R, tag="qx")
                    xb = pqx.tile([128, KH, TCQ], F32R, tag="qx")
                    nc.sync.dma_start(out=xa[:], in_=xT_v[:, 0:KH, t0:t0 + TCQ])
                    nc.sync.dma_start(out=xb[:], in_=xT_v[:, KH:KC, t0:t0 + TCQ])
                    ps = wide.tile([128, HC, TCQ], F32, tag="wide")
                    for m in range(HC):
                        for kc in range(KC):
                            xc = xa if kc < KH else xb
                            nc.tensor.matmul(
                                ps[:, m, :],
                                wq_sb[:, kc, m * 128:(m + 1) * 128],
                                xc[:, kc % KH, :],
                                start=(kc == 0),
                                stop=(kc == KC - 1),
                            )
                    qc = pqst.tile([128, HC, TCQ], F32R, tag="qst")
                    nc.vector.tensor_copy(
                        qc[:].rearrange("p a b -> p (a b)"),
                        ps[:].rearrange("p a b -> p (a b)"),
                    )
                    q_cols[g] = qc

                units = [(b, tqc, h)
                         for b in range(B)
                         for tqc in range(NTQ)
                         for h in range(HC)]
                state = {}
                ot_map = {}
                bias_cur = {}

                def stage_a(i):
                    b, tqc, h = units[i]
                    tg0 = b * S
                    q0 = tqc * TQ
                    ensure_q(b, tqc)
                    if b not in kv_cur:
                        load_kv(b)
                    kb, vb = kv_cur[b]
                    if (b, tqc) not in bias_cur:
                        bc = p2bias.tile([128, NTK, TQ], F32, tag="bias")
                        nc.gpsimd.dma_start(
                            out=bc[:], in_=biasT_v[:, :, q0:q0 + TQ])
                        bias_cur.clear()
                        bias_cur[(b, tqc)] = bc
                    bc_full = bias_cur[(b, tqc)]
                    g = (tg0 + q0) // TCQ
                    qcol = q_cols[g][:, h, :]
                    s_buf = p2s.tile([128, NTK, TQ], F32, tag="s")
                    for tkg in range(NTK // 2):
                        sps = spsp.tile([128, 2, TQ], F32, tag="sps")
                        for tkk in range(2):
                            tkb = tkg * 2 + tkk
                            nc.tensor.matmul(
                                sps[:, tkk, :],
                                kb[:, h, tkb * 128:(tkb + 1) * 128],
                                qcol,
                                start=True,
                                stop=True,
                            )
                        nc.vector.tensor_add(
                            s_buf[:, tkg * 2:(tkg + 1) * 2, :],
                            sps[:],
                            bc_full[:, tkg * 2:(tkg + 1) * 2, :],
                        )
                    state[i] = (s_buf, kv_cur[b])

                def stage_b(i):
                    b, tqc, h = units[i]
                    s_buf, (kb, vb) = state.pop(i)
                    er_buf = p2er.tile([128, NTK, TQ], F32R, tag="er")
                    avp = acc.tile([128, TQ], F32, tag="acc")
                    zb = acc.tile([128, TQ], F32, tag="acc")
                    for half in range(2):
                        hs = slice(half * NHF, (half + 1) * NHF)
                        s_flat = s_buf[:, hs, :].rearrange("p a b -> p (a b)")
                        nc.scalar.activation(
                            s_flat, s_flat,
                            mybir.ActivationFunctionType.Tanh,
                            scale=1.0 / cap,
                        )
                        nc.scalar.activation(
                            er_buf[:, hs, :].rearrange("p a b -> p (a b)"),
                            s_flat,
                            mybir.ActivationFunctionType.Exp,
                            scale=cap,
                        )
                        for tkb in range(half * NHF, (half + 1) * NHF):
                            nc.tensor.matmul(
                                avp[:],
                                vb[:, h, tkb, :],
                                er_buf[:, tkb, :],
                                start=(tkb == 0),
                                stop=(tkb == NTK - 1),
                            )
                            nc.tensor.matmul(
                                zb[:],
                                ones_full[:],
                                er_buf[:, tkb, :],
                                start=(tkb == 0),
                                stop=(tkb == NTK - 1),
                            )
                    recip = p2misc.tile([128, TQ], F32, tag="recip")
                    nc.vector.reciprocal_approx_fast(out=recip[:], in_=zb[:])
                    ot_st = p2ot.tile([128, TQ], F32R, tag="ot")
                    nc.vector.tensor_mul(ot_st[:], avp[:], recip[:])
                    ot_map[(b, tqc, h)] = ot_st

                def phase3_chunks(b, tqc):
                    o0 = ot_map.pop((b, tqc, 0))
                    o1 = ot_map.pop((b, tqc, 1))
                    for tb4 in range(TQ // 128):
                        tb = tqc * (TQ // 128) + tb4
                        for ng in range(2):
                            ps3 = wide.tile([128, 2, 512], F32, tag="wide")
                            for nc2 in range(2):
                                ncol = ng * 2 + nc2
                                for hc, o in ((0, o0), (1, o1)):
                                    nc.tensor.matmul(
                                        ps3[:, nc2, :],
                                        o[:, tb4 * 128:(tb4 + 1) * 128],
                                        wo_sb[:, hc, ncol, :],
                                        start=(hc == 0),
                                        stop=(hc == HC - 1),
                                    )
                            outt = p3out.tile([128, 1024], F32, tag="outt")
                            nc.vector.tensor_copy(
                                outt[:], ps3[:].rearrange("p a b -> p (a b)")
                            )
                            nc.gpsimd.dma_start(
                                out=out_d[b * S + tb * 128:
                                          b * S + (tb + 1) * 128,
                                          ng * 1024:(ng + 1) * 1024],
                                in_=outt[:],
                            )

                stage_a(0)
                for i in range(len(units)):
                    if i + 1 < len(units):
                        stage_a(i + 1)
                    stage_b(i)
                    b, tqc, h = units[i]
                    if h == 1:
                        phase3_chunks(b, tqc)

    nc.compile()
    return nc


_PROGRAM_CACHE: dict = {}


def _get_program(cap: float):
    if cap not in _PROGRAM_CACHE:
        _PROGRAM_CACHE[cap] = _build_program(cap)
    return _PROGRAM_CACHE[cap]


def _prepare_in_maps(inp, wq, wk, wv, wo, attn_bias, softcap):
    x = np.ascontiguousarray(np.asarray(inp, dtype=np.float32)).reshape(T, D)
    xT = _round_fp32r(np.ascontiguousarray(x.T))
    biasT = np.ascontiguousarray(
        np.asarray(attn_bias, dtype=np.float32).reshape(S, S).T
    )
    wq = np.asarray(wq, dtype=np.float32)
    wk = np.asarray(wk, dtype=np.float32)
    wv = np.asarray(wv, dtype=np.float32)
    wo = np.asarray(wo, dtype=np.float32)
    scale = 1.0 / np.sqrt(np.float32(DK))

    in_maps = []
    for c in range(NCORES):
        rows = slice(c * DPC, (c + 1) * DPC)
        in_maps.append({
            "xT": xT,
            "ones": np.ones((128, 128), dtype=np.float32),
            "wqT": _round_fp32r((wq[rows] * scale).T),
            "wkT": _round_fp32r(wk[rows].T),
            "wvT": _round_fp32r(wv[rows].T),
            "woT": _round_fp32r(wo[:, rows].T),
            "biasT": biasT,
        })
    return in_maps


def run(inputs: dict, trace: bool = False):
    """Run the SPMD kernel. Returns (full_output, BassKernelResults)."""
    cap = float(inputs["softcap"])
    nc = _get_program(cap)
    in_maps = _prepare_in_maps(
        inputs["inp"], inputs["wq"], inputs["wk"], inputs["wv"],
        inputs["wo"], inputs["attn_bias"], inputs["softcap"],
    )
    res = run_bass_kernel_spmd(
        nc, in_maps, list(range(NCORES)), trace=trace,
    )
    acc = np.zeros((T, D), dtype=np.float64)
    for c in range(NCORES):
        acc += res.results[c]["out_partial"]
    out = acc.astype(np.float32).reshape(B, S, D)
    return out, res


def kernel(**inputs) -> np.ndarray:
    out, _ = run(inputs, trace=False)
    return out


if __name__ == "__main__":
    rng = np.random.default_rng(0)
    sc = 1.0 / np.sqrt(D)
    inputs = {
        "inp": rng.standard_normal((B, S, D)).astype(np.float32),
        "wq": (rng.standard_normal((D, D)) * sc).astype(np.float32),
        "wk": (rng.standard_normal((D, D)) * sc).astype(np.float32),
        "wv": (rng.standard_normal((D, D)) * sc).astype(np.float32),
        "wo": (rng.standard_normal((D, D)) * sc).astype(np.float32),
        "attn_bias": rng.standard_normal((1, 1, S, S)).astype(np.float32),
        "softcap": 30,
    }
    out = kernel(**inputs)
    print("out", out.shape, out.dtype, float(np.abs(out).max()))

